# revision 16
# baseline (speedup 1.0000x reference)
"""Trainium2 Bass kernel for a dense multi-head attention layer.

Problem (hardcoded shapes):
    hidden_states [2, 2048, 2048] fp32, attention_mask [2,1,1,2048] int32 (all ones),
    Wq/Wk/Wv/Wo [2048, 2048] fp32, biases [2048] fp32 (zeros in practice).
    out = MHA(hidden) with H=16 heads, head_dim=128.

Sharding: 8 cores = 2 batches x 4 head-groups (4 heads per core, tensor
parallel over heads). Each core computes q/k/v projections for its 4 heads,
attention, and a partial output projection; the host sums the 4 partials per
batch.

All matmuls run in bf16 (full PE speed AND fast-weight-load: fp32/fp32r
weights disable FWL, which costs an un-hidden ~107ns LDWEIGHTS per matmul —
measured 330ns/mm fp32r vs ~216ns/mm bf16 for [128x128]x[128,512]).
PSUM accumulation is fp32; softmax statistics are fp32.

Everything is SBUF-resident: hidden^T (8MB bf16) and the weights load once
at the start; q/k/v for all 4 heads are evicted from PSUM straight into
resident SBUF tiles (6MB), so phases 2/3 run with zero input DMA. Layouts
avoid any on-device transpose: the host supplies hidden^T and pre-transposed
weights; scores are computed keys-major (sT = kT^T @ qT) so the PV matmul
consumes exp(sT) directly and produces attn^T, which is exactly the lhsT
layout the output projection wants. Softmax denominators come from DVE adds
over key-blocks + a partition all-reduce; normalization folds into the
PSUM->SBUF eviction of the PV accumulator.
"""
import os
import sys

if "/opt/trn_rl_repo" not in sys.path:
    sys.path.insert(0, "/opt/trn_rl_repo")

# If a previous run crashed the NEFF execution, a fresh NRT open with this
# flag recovers the cores instead of failing with EXEC_UNIT_UNRECOVERABLE.
os.environ.setdefault("NEURON_RT_RESET_CORES", "1")

import numpy as np

B, S, D, H, HD = 2, 2048, 2048, 16, 128
NCORES = 8
GROUPS = 4            # head-groups == cores per batch
GH = H // GROUPS      # heads per core = 4
GD = GH * HD          # 512 projection cols per core
ST = 512              # s/q/o tile width
NSB = S // 128        # 16 s-blocks
NEB = D // 128        # 16 e-blocks (contraction)
NST = S // ST         # 4 s-tiles
SCALE = 1.0 / float(np.sqrt(HD))

_RUNNER = None


def _bf16_np():
    from concourse import mybir
    return mybir.dt.np(mybir.dt.bfloat16)


def _build_nc():
    import concourse.tile as tile
    import concourse.bass_isa as bass_isa
    from concourse import bacc, mybir

    phases = os.environ.get("K_PHASES", "123")
    # scheduling toggles; defaults = fastest measured config on HW
    # (es bf16-pair chain + merge on DVE, out stores and PSUM evictions on
    # the ACT ring, PV two score-pairs behind exp, output proj interleaved)
    esmerge = os.environ.get("K_ESMERGE", "dve")    # pool|dve
    outq = os.environ.get("K_OUTQ", "scalar")       # sync|scalar
    evq = os.environ.get("K_EVQ", "scalar")         # vector|scalar
    pvdelay = int(os.environ.get("K_PVDELAY", "2"))  # 0|2
    ilv = os.environ.get("K_ILV", "1") == "1"       # interleave ph3
    es_mode = os.environ.get("K_ES", "bf16pair")    # bf16pair|f32
    repeat = int(os.environ.get("K_REPEAT", "1"))   # body repetitions

    f32 = mybir.dt.float32
    bf16 = mybir.dt.bfloat16
    Exp = mybir.ActivationFunctionType.Exp

    nc = bacc.Bacc("TRN2", target_bir_lowering=False, debug=False,
                   num_devices=NCORES)

    # hT: hidden^T chunked [NEB, 128, S]; w*P: eb-pair-packed [NEB/2,128,2*GD]
    # (2KB DMA lines); woP: [GH, 128, D] (4KB lines). All bf16.
    hT = nc.dram_tensor("hT", [NEB, 128, S], bf16, kind="ExternalInput")
    wqP = nc.dram_tensor("wqP", [NEB // 2, 128, 2 * GD], bf16,
                         kind="ExternalInput")
    wkP = nc.dram_tensor("wkP", [NEB // 2, 128, 2 * GD], bf16,
                         kind="ExternalInput")
    wvP = nc.dram_tensor("wvP", [NEB // 2, 128, 2 * GD], bf16,
                         kind="ExternalInput")
    woP = nc.dram_tensor("woP", [GH, 128, D], bf16, kind="ExternalInput")
    out = nc.dram_tensor("out", [S, D], f32, kind="ExternalOutput")

    with tile.TileContext(nc) as tc:
      for _rep in range(repeat):
        with tc.tile_pool(name="persist", bufs=1) as persist:
            # resident q/k/v (bf16, 6MB) + wo (2MB)
            qt_r = [persist.tile([128, S], bf16, name=f"qtr{h}")
                    for h in range(GH)]
            kt_r = [persist.tile([128, S], bf16, name=f"ktr{h}")
                    for h in range(GH)]
            vt_r = [persist.tile([128, NSB, 128], bf16, name=f"vtr{h}")
                    for h in range(GH)]
            wo_sb = persist.tile([128, GH, D], bf16, name="wosb")

            # ---------------- phase 1: q/k/v projections ----------------
            if "1" in phases:
              with tc.tile_pool(name="hres", bufs=1) as hres, \
                 tc.tile_pool(name="wqk", bufs=1) as wqk, \
                 tc.tile_pool(name="ps1", bufs=8, space="PSUM") as ps1:
                h_sb = hres.tile([128, NEB, S], bf16, name="hsb")
                wq_sb = wqk.tile([128, NEB, GD], bf16, name="wqsb")
                wk_sb = wqk.tile([128, NEB, GD], bf16, name="wksb")
                wv_sb = wqk.tile([128, NEB, GD], bf16, name="wvsb")

                # interleave so eb=0 pieces of the q/k path arrive first
                for i in range(NEB // 2):
                    for j in range(2):
                        nc.sync.dma_start(out=h_sb[:, 2 * i + j, :],
                                          in_=hT[2 * i + j])
                    nc.sync.dma_start(
                        out=wq_sb[:, 2 * i:2 * i + 2, :],
                        in_=wqP[i].rearrange("p (j d) -> p j d", j=2))
                    nc.sync.dma_start(
                        out=wk_sb[:, 2 * i:2 * i + 2, :],
                        in_=wkP[i].rearrange("p (j d) -> p j d", j=2))
                for i in range(NEB // 2):
                    nc.sync.dma_start(
                        out=wv_sb[:, 2 * i:2 * i + 2, :],
                        in_=wvP[i].rearrange("p (j d) -> p j d", j=2))
                for cb in range(GH):
                    nc.sync.dma_start(out=wo_sb[:, cb, :], in_=woP[cb])

                for st in range(NST):
                    ssl = slice(st * ST, (st + 1) * ST)
                    if st == 0:
                        # eb-outer over 8 live accumulators: consume input
                        # chunks in arrival order so the PE tracks the DMA
                        # stream instead of stalling per accumulation.
                        pss = {}
                        for h in range(GH):
                            for t in range(2):
                                pss[(h, t)] = ps1.tile([128, ST], f32,
                                                       tag="ps1",
                                                       name=f"psqk{h}{t}")
                        for eb in range(NEB):
                            for h in range(GH):
                                for t, w_sb in ((0, wq_sb), (1, wk_sb)):
                                    nc.tensor.matmul(
                                        pss[(h, t)],
                                        w_sb[:, eb, h * HD:(h + 1) * HD],
                                        h_sb[:, eb, ssl],
                                        start=(eb == 0), stop=(eb == NEB - 1))
                        for h in range(GH):
                            for t, dst in ((0, qt_r), (1, kt_r)):
                                nc.scalar.copy(dst[h][:, ssl], pss[(h, t)])
                    else:
                        for h in range(GH):
                            for w_sb, dst in ((wq_sb, qt_r), (wk_sb, kt_r)):
                                ps = ps1.tile([128, ST], f32, tag="ps1")
                                for eb in range(NEB):
                                    nc.tensor.matmul(
                                        ps,
                                        w_sb[:, eb, h * HD:(h + 1) * HD],
                                        h_sb[:, eb, ssl],
                                        start=(eb == 0), stop=(eb == NEB - 1))
                                nc.scalar.copy(dst[h][:, ssl], ps)
                    for j in range(ST // 128):
                        ps = ps1.tile([128, GD], f32, tag="ps1")
                        for eb in range(NEB):
                            nc.tensor.matmul(
                                ps,
                                h_sb[:, eb, st * ST + j * 128:
                                     st * ST + (j + 1) * 128],
                                wv_sb[:, eb, :],
                                start=(eb == 0), stop=(eb == NEB - 1))
                        for h in range(GH):
                            nc.scalar.copy(vt_r[h][:, st * 4 + j, :],
                                           ps[:, h * HD:(h + 1) * HD])

            # ------- phase 2+3: attention fused with output projection -------
            # qt outer / heads inner. Per (qt, h) "slot" the PE does 48 MMs
            # (16 scores + 16 PV + 16 interleaved output-projection); ACT does
            # the exp stream (+reciprocal), DVE the bf16 softmax-sum chain,
            # po evictions and the normalize-multiply, Pool the merge +
            # partition reduce, SP the output DMA. PV runs two score-pairs
            # behind so the ACT exp latency is fully hidden; the output
            # projection of query tile qt-1 is interleaved 2 MMs per score
            # pair to fill the same bubbles.
            if "2" in phases:
              with tc.tile_pool(name="expp", bufs=3) as expp, \
                 tc.tile_pool(name="sm", bufs=2) as sm, \
                 tc.tile_pool(name="attn2", bufs=2) as attn2, \
                 tc.tile_pool(name="ev3", bufs=3) as ev3, \
                 tc.tile_pool(name="ps_s", bufs=2, space="PSUM") as ps_s, \
                 tc.tile_pool(name="pap", bufs=2, space="PSUM") as pap, \
                 tc.tile_pool(name="pop", bufs=2, space="PSUM") as pop:

                def ph3_ops(prev, qt_prev, j):
                    # micro-op closures: 16 matmuls + evictions + 1 store for
                    # the output projection of block (qt_prev, j)
                    sb = qt_prev * (ST // 128) + j
                    ov = ev3.tile([128, D], f32, tag="ov")
                    state = {}

                    def mk(ot, cb):
                        def op():
                            if cb == 0:
                                state["po"] = pop.tile([128, ST], f32,
                                                       tag="po", name="po")
                            nc.tensor.matmul(
                                state["po"],
                                prev[cb][:, j * 128:(j + 1) * 128],
                                wo_sb[:, cb, ot * ST:(ot + 1) * ST],
                                start=(cb == 0), stop=(cb == GH - 1))
                            if cb == GH - 1:
                                if evq == "vector":
                                    nc.vector.tensor_copy(
                                        ov[:, ot * ST:(ot + 1) * ST],
                                        state["po"])
                                else:
                                    nc.scalar.copy(
                                        ov[:, ot * ST:(ot + 1) * ST],
                                        state["po"])
                                if ot == NST - 1:
                                    dq = (nc.sync if outq == "sync"
                                          else nc.scalar)
                                    dq.dma_start(
                                        out=out[sb * 128:(sb + 1) * 128, :],
                                        in_=ov)
                        return op

                    return [mk(ot, cb) for ot in range(NST)
                            for cb in range(GH)]

                def pv_pair(h, kb2, expt_halves, pa):
                    expt = expt_halves[kb2 // 4]
                    kbo = (kb2 % 4) * 2
                    for half in range(2):
                        kb = kb2 * 2 + half
                        nc.tensor.matmul(pa, vt_r[h][:, kb, :],
                                         expt[:, kbo + half, :],
                                         start=(kb == 0),
                                         stop=(kb == NSB - 1))

                prev_attn = None
                for qt in range(NST):
                    attn_t = []
                    for h in range(GH):
                        pending = (ph3_ops(prev_attn, qt - 1, h)
                                   if prev_attn is not None else [])
                        qs = qt_r[h][:, qt * ST:(qt + 1) * ST]
                        expt_halves = [
                            expp.tile([128, NSB // 2, ST], bf16, tag="expt",
                                      name=f"expt{h}{half}")
                            for half in range(2)]
                        if es_mode == "bf16pair":
                            es2 = sm.tile([128, 2, ST], bf16, tag="es2")
                        else:
                            es2 = sm.tile([128, ST], f32, tag="esf")
                        pa = pap.tile([128, ST], f32, tag="pa")
                        for kb2 in range(NSB // 2):
                            expt = expt_halves[kb2 // 4]
                            kbo = (kb2 % 4) * 2
                            ps = ps_s.tile([128, 2, ST], f32, tag="ps")
                            for half in range(2):
                                kb = kb2 * 2 + half
                                nc.tensor.matmul(
                                    ps[:, half, :],
                                    kt_r[h][:, kb * 128:(kb + 1) * 128],
                                    qs, start=True, stop=True)
                            pair = expt[:, kbo:kbo + 2, :]
                            nc.scalar.activation(pair, ps, Exp, scale=SCALE)
                            if es_mode == "bf16pair":
                                if kb2 == 0:
                                    nc.vector.tensor_copy(es2, pair)
                                else:
                                    nc.vector.tensor_add(es2, es2, pair)
                            else:
                                if kb2 == 0:
                                    nc.vector.tensor_copy(
                                        es2, expt[:, kbo, :])
                                else:
                                    nc.vector.tensor_add(
                                        es2, es2, expt[:, kbo, :])
                                nc.vector.tensor_add(
                                    es2, es2, expt[:, kbo + 1, :])
                            if ilv:
                                for _ in range(2):
                                    if pending:
                                        pending.pop(0)()
                            if kb2 >= pvdelay:
                                pv_pair(h, kb2 - pvdelay, expt_halves, pa)
                        for kb2 in range(NSB // 2 - pvdelay, NSB // 2):
                            pv_pair(h, kb2, expt_halves, pa)
                        while pending:
                            pending.pop(0)()
                        if es_mode == "bf16pair":
                            es = sm.tile([128, ST], f32, tag="es")
                            if esmerge == "pool":
                                nc.gpsimd.tensor_add(es, es2[:, 0, :],
                                                     es2[:, 1, :])
                            else:
                                nc.vector.tensor_add(es, es2[:, 0, :],
                                                     es2[:, 1, :])
                        else:
                            es = es2
                        bcsum = sm.tile([128, ST], f32, tag="bcsum")
                        nc.gpsimd.partition_all_reduce(
                            bcsum, es, 128, bass_isa.ReduceOp.add)
                        brc = sm.tile([128, ST], f32, tag="brc")
                        nc.vector.reciprocal(brc, bcsum)
                        at = attn2.tile([128, ST], bf16, name=f"at{h}",
                                        tag=f"at{h}")
                        nc.vector.tensor_mul(at, pa, brc)
                        attn_t.append(at)
                    prev_attn = attn_t
                for j in range(ST // 128):
                    for op in ph3_ops(prev_attn, NST - 1, j):
                        op()
            else:
                with tc.tile_pool(name="zf", bufs=1) as zf:
                    z = zf.tile([128, D], f32)
                    nc.vector.memset(z, 0.0)
                    for sb in range(NSB):
                        nc.scalar.dma_start(out=out[sb * 128:(sb + 1) * 128, :],
                                            in_=z)

    nc.compile()
    return nc


def _get_runner():
    global _RUNNER
    if _RUNNER is None:
        _RUNNER = _build_nc()
    return _RUNNER


def _prepare_in_maps(hidden_states, Wq, Wk, Wv, Wo):
    bf16 = _bf16_np()
    hidden = np.asarray(hidden_states, dtype=np.float32)
    # [D, S] -> chunked [NEB, 128, S]
    hT = [np.ascontiguousarray(hidden[b].T).astype(bf16).reshape(NEB, 128, S)
          for b in range(B)]
    wq = np.asarray(Wq, dtype=np.float32)
    wk = np.asarray(Wk, dtype=np.float32)
    wv = np.asarray(Wv, dtype=np.float32)
    wo = np.asarray(Wo, dtype=np.float32)

    def pack_w(w, rows):
        # w[rows, :].T is [D, GD]; pack eb-pairs -> [NEB/2, 128, 2*GD]
        wT = np.ascontiguousarray(w[rows, :].T).astype(bf16)
        return np.ascontiguousarray(
            wT.reshape(NEB // 2, 2, 128, GD).transpose(0, 2, 1, 3)
        ).reshape(NEB // 2, 128, 2 * GD)

    in_maps = []
    for core in range(NCORES):
        b, g = divmod(core, GROUPS)
        rows = slice(g * GD, (g + 1) * GD)
        woT = np.ascontiguousarray(wo[:, rows].T).astype(bf16)
        in_maps.append({
            "hT": hT[b],
            "wqP": pack_w(wq, rows),
            "wkP": pack_w(wk, rows),
            "wvP": pack_w(wv, rows),
            "woP": woT.reshape(GH, 128, D),
        })
    return in_maps


def _run_device(in_maps, trace=False):
    from concourse.bass_utils import run_bass_kernel_spmd
    nc = _get_runner()
    try:
        return run_bass_kernel_spmd(nc, in_maps, core_ids=list(range(NCORES)),
                                    trace=trace)
    except Exception:
        # Transient device failures (rare) are recoverable by reopening the
        # backend with NEURON_RT_RESET_CORES=1. Retry once.
        try:
            import jax
            jax.clear_caches()
            try:
                jax.extend.backend.clear_backends()
            except Exception:
                jax._src.api.clear_backends()
        except Exception:
            pass
        return run_bass_kernel_spmd(nc, in_maps, core_ids=list(range(NCORES)),
                                    trace=trace)


def _numpy_reference(hidden_states, attention_mask, Wq, bq, Wk, bk, Wv, bv,
                     Wo, bo):
    """Exact fallback for inputs the fast path does not handle."""
    h = np.asarray(hidden_states, dtype=np.float32)
    mask = np.asarray(attention_mask)
    q = h @ np.asarray(Wq, np.float32).T + np.asarray(bq, np.float32)
    k = h @ np.asarray(Wk, np.float32).T + np.asarray(bk, np.float32)
    v = h @ np.asarray(Wv, np.float32).T + np.asarray(bv, np.float32)
    q = q.reshape(B, S, H, HD).transpose(0, 2, 1, 3)
    k = k.reshape(B, S, H, HD).transpose(0, 2, 1, 3)
    v = v.reshape(B, S, H, HD).transpose(0, 2, 1, 3)
    scores = (q @ k.transpose(0, 1, 3, 2)).astype(np.float32) * SCALE
    scores = np.where(mask == 0, np.float32(-1e9), scores)
    scores -= scores.max(axis=-1, keepdims=True)
    probs = np.exp(scores, dtype=np.float32)
    probs /= probs.sum(axis=-1, keepdims=True)
    attn = probs @ v
    attn = attn.transpose(0, 2, 1, 3).reshape(B, S, D)
    out = attn @ np.asarray(Wo, np.float32).T + np.asarray(bo, np.float32)
    return out.astype(np.float32)


def kernel(hidden_states, attention_mask, Wq, bq, Wk, bk, Wv, bv, Wo, bo):
    mask = np.asarray(attention_mask)
    bq_np = np.asarray(bq, dtype=np.float32)
    if (mask == 0).any() or np.any(bq_np):
        # general (never hit with the reference setup_inputs): bq shifts
        # scores per-key and a masked key changes the softmax support —
        # neither is representable in the fast path's fused layout.
        return _numpy_reference(hidden_states, attention_mask, Wq, bq, Wk,
                                bk, Wv, bv, Wo, bo)

    in_maps = _prepare_in_maps(hidden_states, Wq, Wk, Wv, Wo)
    res = _run_device(in_maps)

    # bk only adds a per-query constant to scores (softmax-invariant).
    # bv passes through the probs (rows sum to 1): out += bv @ Wo.T. bo adds.
    extra = (np.asarray(bv, np.float64) @ np.asarray(Wo, np.float64).T
             + np.asarray(bo, np.float64))
    out = np.empty((B, S, D), dtype=np.float32)
    for b in range(B):
        acc = np.zeros((S, D), dtype=np.float64)
        for g in range(GROUPS):
            acc += res.results[b * GROUPS + g]["out"]
        out[b] = (acc + extra).astype(np.float32)
    return out
